# revision 5
# baseline (speedup 1.0000x reference)
"""Self pairwise Euclidean distance on Trainium2 (8 NeuronCores).

out[i, j] = ||x[j] - x[i]||_2 for x of shape [8192, 64] fp32.

Sharding + symmetry: core c owns rows [1024c, 1024(c+1)). For each
128-row tile m it computes only a 4224-column circular band of d^2
(rotated start s0 = 128m), since rows [128m, 128m+128) need columns
[r+1, r+4096] (mod N) and every unordered pair {i, j} lands in at least
one band: delta = (j-i) mod N <= 4096 is covered directly, larger delta
at the transposed entry. The host mirrors the other triangle from the
transpose, takes sqrt, and pins the diagonal — the device moves ~27%
of the naive full-matrix bytes.

Device pipeline (per core, identical SPMD program):
  * d^2 comes straight out of ONE matmul per 512-col chunk via a K=68
    augmented contraction in bf16: A = [x^T; 1; 1; sqn_hi; sqn_lo],
    B = [-2 x^T; sqn_hi; sqn_lo; 1; 1] => psum = d2 exactly. The norms
    ride in two bf16 rows (hi + lo residual) so their quantization does
    not leak into d^2; all operand prep happens on the host.
  * PSUM groups are 1024 cols (2 banks) from a 4-deep pool = all 8
    banks, so up to 4 groups are in flight and the PSUM->SBUF drain
    never stalls the matmuls.
  * The drain (the only elementwise device work, fp32 PSUM -> fp16
    SBUF) alternates ScalarE Copy / VectorE tensor_copy per group, so
    both engines share it and neither paces the stream.
  * fp16 d^2 bands DMA out at ~3us per band; the stream saturates the
    DMA budget from ~10us to the end.
  * Ramp: B's head loads via SP/HWDGE while A's first row tile, B's
    tail and the rest of A ride Pool/SWDGE (independent generation
    queues); band 0 uses small leading groups and bands 0-1 issue
    per-group output DMAs; a 13-matmul warm-up stream on zeroed scratch
    keeps the PE busy so real matmuls are costed at the ramped clock.
"""

import os

import numpy as np

N = 8192
D = 64
K = D + 4  # x rows + [ones, ones] + [sqn_hi, sqn_lo]
NCORES = 8
RPC = N // NCORES  # rows per core
PT = 128  # output partition tile (rows per matmul)
NT_M = RPC // PT  # 8 row tiles per core
LB = 4224  # band columns per row tile (127 + 4096 + 1)
GRP = 1024  # psum group cols (2 banks)
BLOAD = 128 * (NT_M - 1) + LB  # 5120 rotated B columns a core ever touches
SPLIT = 2048  # band cols in the first output DMA (bands >= 1)

_NC_CACHE = {}


def _band_start(m: int) -> int:
    return 128 * m


def _group_sizes():
    sizes = [GRP] * 4
    if LB > 4 * GRP:
        sizes.append(LB - 4 * GRP)
    return sizes


def _build_nc(mm_dtype_name: str):
    import concourse.mybir as mybir
    import concourse.tile as tile
    from concourse import bacc

    f32 = mybir.dt.float32
    f16 = mybir.dt.float16
    mm_dt = getattr(mybir.dt, mm_dtype_name)
    AF = mybir.ActivationFunctionType

    nc = bacc.Bacc(
        "TRN2",
        target_bir_lowering=False,
        debug=False,
        num_devices=NCORES,
    )
    bt = nc.dram_tensor("bt", [K, BLOAD], mm_dt, kind="ExternalInput").ap()
    at = nc.dram_tensor("at", [K, RPC], mm_dt, kind="ExternalInput").ap()
    out = nc.dram_tensor("out", [RPC, LB], f16, kind="ExternalOutput").ap()

    with tile.TileContext(nc) as tc:
        with (
            tc.tile_pool(name="persist", bufs=1) as persist,
            tc.tile_pool(name="band", bufs=3) as bandp,
            tc.tile_pool(name="ps", bufs=4, space="PSUM") as psp,
        ):
            B = persist.tile([K, BLOAD], mm_dt)
            A = persist.tile([K, RPC], mm_dt)
            W = persist.tile([K, 512], mm_dt)  # PE warm-up scratch

            # Few, big input DMAs (per-DMA issue overhead paces the stream
            # more than transfer time): B's head rides SP/HWDGE while A's
            # first row tile, B's tail and the rest of A ride Pool/SWDGE,
            # so the two generation queues run concurrently and the first
            # matmul's operands both land ~2.4us in.
            nc.sync.dma_start(B[:, 0:1024], bt[:, 0:1024])
            nc.sync.dma_start(B[:, 1024:2048], bt[:, 1024:2048])
            nc.gpsimd.dma_start(A[:, 0:PT], at[:, 0:PT])
            nc.gpsimd.dma_start(B[:, 2048:BLOAD], bt[:, 2048:BLOAD])
            nc.gpsimd.dma_start(A[:, PT:RPC], at[:, PT:RPC])

            # PE warm-up scratch source (see the g==0 block below).
            nc.vector.memset(W[:, :], 0.0)

            cvt = 0
            for m in range(NT_M):
                s0 = _band_start(m)
                band = bandp.tile([PT, LB], f16)
                # Band 0 ramps the pipeline: small leading groups so the
                # first convert (and first output DMA) fire early.
                if m == 0:
                    sizes = [256, 256, 512] + [GRP] * 3 + [LB - 4 * GRP]
                else:
                    sizes = _group_sizes()
                off = 0
                for g, sz in enumerate(sizes):
                    ps = psp.tile([PT, GRP], f32)
                    if m == 0 and g == 0:
                        # PE warm-up: a stream of tiny matmuls on zeroed
                        # scratch keeps the tensor engine continuously busy
                        # from ~1.3us, so the real matmuls (queued behind in
                        # PE FIFO order) are all costed at the full ramped
                        # clock. They write a psum region no convert reads.
                        for _ in range(13):
                            nc.tensor.matmul(
                                ps[:, 512 : 512 + PT],
                                W[:, 0:PT],
                                W[:, 0:PT],
                                start=True,
                                stop=True,
                            )
                    for j in range(0, sz, 512):
                        w = min(512, sz - j)
                        col = s0 + off + j
                        nc.tensor.matmul(
                            ps[:, j : j + w],
                            A[:, m * PT : (m + 1) * PT],
                            B[:, col : col + w],
                            start=True,
                            stop=True,
                        )
                    dst = band[:, off : off + sz]
                    src = ps[:, 0:sz]
                    use_act = cvt % 2 == 0
                    if use_act:
                        nc.scalar.activation(dst, src, AF.Copy)
                    else:
                        nc.vector.tensor_copy(dst, src)
                    cvt += 1
                    off += sz
                    rows = slice(m * PT, (m + 1) * PT)
                    if m <= 1 and off <= 3 * GRP:
                        # Fine-grained DMAs so the output stream starts at
                        # the first convert. SP's config queue is short (2
                        # input DMAs), so SP issues these without the extra
                        # in-order config delay the ACT queue would add
                        # behind its own converts.
                        nc.sync.dma_start(
                            out[rows, off - sz : off], band[:, off - sz : off]
                        )
                    elif m <= 1 and off == LB:
                        nc.sync.dma_start(
                            out[rows, 3 * GRP : LB], band[:, 3 * GRP : LB]
                        )
                    elif off == SPLIT:
                        nc.sync.dma_start(out[rows, 0:SPLIT], band[:, 0:SPLIT])
                    elif off == LB:
                        nc.sync.dma_start(out[rows, SPLIT:LB], band[:, SPLIT:LB])
    nc.compile()
    return nc


def _get_nc():
    mm_dtype = os.environ.get("KERNEL_MM_DTYPE", "bfloat16")
    if mm_dtype not in _NC_CACHE:
        _NC_CACHE[mm_dtype] = _build_nc(mm_dtype)
    return _NC_CACHE[mm_dtype]


def _round_bf16(a: np.ndarray) -> np.ndarray:
    """Round fp32 to the bf16 grid (round-to-nearest-even), keep fp32."""
    u = np.ascontiguousarray(a, dtype=np.float32).view(np.uint32)
    r = (u + np.uint32(0x7FFF) + ((u >> np.uint32(16)) & np.uint32(1))) & np.uint32(
        0xFFFF0000
    )
    return r.view(np.float32)


def _to_bf16(a: np.ndarray):
    import ml_dtypes

    return np.ascontiguousarray(a).astype(ml_dtypes.bfloat16)


def _host_inputs(x: np.ndarray) -> list[dict]:
    xb = _round_bf16(x)  # [N, D] on the bf16 grid
    xt = np.ascontiguousarray(xb.T)  # [D, N]
    sqn = (xb.astype(np.float64) ** 2).sum(1).astype(np.float32)
    sqn_hi = _round_bf16(sqn)
    sqn_lo = _round_bf16(sqn - sqn_hi)
    ones = np.ones((1, N), np.float32)
    # -2*xb is exact (exponent shift), stays on the bf16 grid.
    b_full = np.concatenate(
        [-2.0 * xt, sqn_hi[None, :], sqn_lo[None, :], ones, ones], axis=0
    )  # [K, N]
    a_full = np.concatenate(
        [xt, ones, ones, sqn_hi[None, :], sqn_lo[None, :]], axis=0
    )  # [K, N]
    in_maps = []
    for c in range(NCORES):
        r0 = c * RPC
        bt = np.concatenate([b_full[:, r0:], b_full[:, :r0]], axis=1)[:, :BLOAD]
        in_maps.append(
            {
                "bt": _to_bf16(bt),
                "at": _to_bf16(a_full[:, r0 : r0 + RPC]),
            }
        )
    return in_maps


def _assemble(blocks: list[np.ndarray]) -> np.ndarray:
    """blocks[c]: [RPC, LB] fp16 d^2 bands -> full [N, N] fp32 distances."""
    full = np.empty((N, N), np.float32)
    HB = 128  # rows sharing one band start (1 row tile)
    for c in range(NCORES):
        blk = np.sqrt(np.maximum(blocks[c].astype(np.float32), 0.0))
        for half in range(RPC // HB):
            r0 = c * RPC + half * HB  # band's global col start == r0
            data = blk[half * HB : (half + 1) * HB]
            seg = min(LB, N - r0)
            full[r0 : r0 + HB, r0 : r0 + seg] = data[:, :seg]
            if seg < LB:
                full[r0 : r0 + HB, : LB - seg] = data[:, seg:]
    # Mirror the uncovered circular segment of each HB-row block from the
    # transpose (those entries are always device-covered).
    for b in range(N // HB):
        r0 = b * HB
        s = (r0 + LB) % N
        e = s + (N - LB)
        if e <= N:
            full[r0 : r0 + HB, s:e] = full[s:e, r0 : r0 + HB].T
        else:
            full[r0 : r0 + HB, s:N] = full[s:N, r0 : r0 + HB].T
            full[r0 : r0 + HB, : e - N] = full[: e - N, r0 : r0 + HB].T
    np.fill_diagonal(full, 0.0)
    return full


def _run(inputs, trace=False, trace_cores=None):
    from concourse.bass_utils import run_bass_kernel_spmd

    x = np.ascontiguousarray(np.asarray(inputs["x"], dtype=np.float32))
    assert x.shape == (N, D), x.shape
    res = run_bass_kernel_spmd(
        _get_nc(),
        _host_inputs(x),
        core_ids=list(range(NCORES)),
        trace=trace,
        trace_cores=trace_cores,
    )
    full = _assemble([r["out"] for r in res.results])
    return full, res


def kernel(**inputs) -> np.ndarray:
    full, _ = _run(inputs)
    return full


# revision 6
# speedup vs baseline: 1.0031x; 1.0031x over previous
"""Self pairwise Euclidean distance on Trainium2 (8 NeuronCores).

out[i, j] = ||x[j] - x[i]||_2 for x of shape [8192, 64] fp32.

Sharding + symmetry: core c owns rows [1024c, 1024(c+1)). For each
128-row tile m it computes only a 4224-column circular band of d^2
(rotated start s0 = 128m), since rows [128m, 128m+128) need columns
[r+1, r+4096] (mod N) and every unordered pair {i, j} lands in at least
one band: delta = (j-i) mod N <= 4096 is covered directly, larger delta
at the transposed entry. The host mirrors the other triangle from the
transpose, takes sqrt, and pins the diagonal — the device moves ~27%
of the naive full-matrix bytes.

Device pipeline (per core, identical SPMD program):
  * d^2 comes straight out of ONE matmul per 512-col chunk via a K=68
    augmented contraction in bf16: A = [x^T; 1; 1; sqn_hi; sqn_lo],
    B = [-2 x^T; sqn_hi; sqn_lo; 1; 1] => psum = d2 exactly. The norms
    ride in two bf16 rows (hi + lo residual) so their quantization does
    not leak into d^2; all operand prep happens on the host.
  * PSUM groups are 1024 cols (2 banks) from a 4-deep pool = all 8
    banks, so up to 4 groups are in flight and the PSUM->SBUF drain
    never stalls the matmuls.
  * The drain (the only elementwise device work, fp32 PSUM -> fp16
    SBUF) alternates ScalarE Copy / VectorE tensor_copy per group, so
    both engines share it and neither paces the stream.
  * fp16 d^2 bands DMA out at ~3us per band; the stream saturates the
    DMA budget from ~10us to the end.
  * Ramp: B's head loads via SP/HWDGE while A's first row tile, B's
    tail and the rest of A ride Pool/SWDGE (independent generation
    queues); band 0 uses small leading groups and bands 0-1 issue
    per-group output DMAs; a 13-matmul warm-up stream on zeroed scratch
    keeps the PE busy so real matmuls are costed at the ramped clock.
"""

import os

import numpy as np

N = 8192
D = 64
K = D + 4  # x rows + [ones, ones] + [sqn_hi, sqn_lo]
NCORES = 8
RPC = N // NCORES  # rows per core
PT = 128  # output partition tile (rows per matmul)
NT_M = RPC // PT  # 8 row tiles per core
LB = 4224  # band columns per row tile (127 + 4096 + 1)
GRP = 1024  # psum group cols (2 banks)
BLOAD = 128 * (NT_M - 1) + LB  # 5120 rotated B columns a core ever touches
SPLIT = 2048  # band cols in the first output DMA (bands >= 1)

_NC_CACHE = {}


def _band_start(m: int) -> int:
    return 128 * m


def _group_sizes():
    sizes = [GRP] * 4
    if LB > 4 * GRP:
        sizes.append(LB - 4 * GRP)
    return sizes


def _build_nc(mm_dtype_name: str):
    import concourse.mybir as mybir
    import concourse.tile as tile
    from concourse import bacc

    f32 = mybir.dt.float32
    f16 = mybir.dt.float16
    mm_dt = getattr(mybir.dt, mm_dtype_name)
    AF = mybir.ActivationFunctionType

    nc = bacc.Bacc(
        "TRN2",
        target_bir_lowering=False,
        debug=False,
        num_devices=NCORES,
    )
    bt = nc.dram_tensor("bt", [K, BLOAD], mm_dt, kind="ExternalInput").ap()
    at = nc.dram_tensor("at", [K, RPC], mm_dt, kind="ExternalInput").ap()
    out = nc.dram_tensor("out", [RPC, LB], f16, kind="ExternalOutput").ap()

    with tile.TileContext(nc) as tc:
        with (
            tc.tile_pool(name="persist", bufs=1) as persist,
            tc.tile_pool(name="band", bufs=3) as bandp,
            tc.tile_pool(name="ps", bufs=4, space="PSUM") as psp,
        ):
            B = persist.tile([K, BLOAD], mm_dt)
            A = persist.tile([K, RPC], mm_dt)
            W = persist.tile([K, 512], mm_dt)  # PE warm-up scratch

            # Few, big input DMAs (per-DMA issue overhead paces the stream
            # more than transfer time): B's head rides SP/HWDGE while A's
            # first row tile, B's tail and the rest of A ride Pool/SWDGE,
            # so the two generation queues run concurrently and the first
            # matmul's operands both land ~2.4us in.
            nc.sync.dma_start(B[:, 0:1024], bt[:, 0:1024])
            nc.sync.dma_start(B[:, 1024:2048], bt[:, 1024:2048])
            nc.gpsimd.dma_start(A[:, 0:PT], at[:, 0:PT])
            nc.gpsimd.dma_start(B[:, 2048:BLOAD], bt[:, 2048:BLOAD])
            nc.gpsimd.dma_start(A[:, PT:RPC], at[:, PT:RPC])

            # PE warm-up scratch source (see the g==0 block below).
            nc.vector.memset(W[:, :], 0.0)

            cvt = 0
            for m in range(NT_M):
                s0 = _band_start(m)
                band = bandp.tile([PT, LB], f16)
                # Band 0 ramps the pipeline: small leading groups so the
                # first convert (and first output DMA) fire early.
                if m == 0:
                    sizes = [256, 256, 512] + [GRP] * 3 + [LB - 4 * GRP]
                else:
                    sizes = _group_sizes()
                off = 0
                for g, sz in enumerate(sizes):
                    ps = psp.tile([PT, GRP], f32)
                    if m == 0 and g == 0:
                        # PE warm-up: a stream of tiny matmuls on zeroed
                        # scratch keeps the tensor engine continuously busy
                        # from ~1.3us, so the real matmuls (queued behind in
                        # PE FIFO order) are all costed at the full ramped
                        # clock. They write a psum region no convert reads.
                        for _ in range(13):
                            nc.tensor.matmul(
                                ps[:, 512 : 512 + PT],
                                W[:, 0:PT],
                                W[:, 0:PT],
                                start=True,
                                stop=True,
                            )
                    for j in range(0, sz, 512):
                        w = min(512, sz - j)
                        col = s0 + off + j
                        nc.tensor.matmul(
                            ps[:, j : j + w],
                            A[:, m * PT : (m + 1) * PT],
                            B[:, col : col + w],
                            start=True,
                            stop=True,
                        )
                    dst = band[:, off : off + sz]
                    src = ps[:, 0:sz]
                    # DVE takes the even converts: it is free immediately,
                    # while ACT spends the first ~2us on its table load.
                    use_act = cvt % 2 == 1
                    if use_act:
                        nc.scalar.activation(dst, src, AF.Copy)
                    else:
                        nc.vector.tensor_copy(dst, src)
                    cvt += 1
                    off += sz
                    rows = slice(m * PT, (m + 1) * PT)
                    if m <= 1 and off <= 3 * GRP:
                        # Fine-grained DMAs so the output stream starts at
                        # the first convert. SP's config queue is short (2
                        # input DMAs), so SP issues these without the extra
                        # in-order config delay the ACT queue would add
                        # behind its own converts.
                        nc.sync.dma_start(
                            out[rows, off - sz : off], band[:, off - sz : off]
                        )
                    elif m <= 1 and off == LB:
                        nc.sync.dma_start(
                            out[rows, 3 * GRP : LB], band[:, 3 * GRP : LB]
                        )
                    elif off == SPLIT:
                        nc.sync.dma_start(out[rows, 0:SPLIT], band[:, 0:SPLIT])
                    elif off == LB:
                        nc.sync.dma_start(out[rows, SPLIT:LB], band[:, SPLIT:LB])
    nc.compile()
    return nc


def _get_nc():
    mm_dtype = os.environ.get("KERNEL_MM_DTYPE", "bfloat16")
    if mm_dtype not in _NC_CACHE:
        _NC_CACHE[mm_dtype] = _build_nc(mm_dtype)
    return _NC_CACHE[mm_dtype]


def _round_bf16(a: np.ndarray) -> np.ndarray:
    """Round fp32 to the bf16 grid (round-to-nearest-even), keep fp32."""
    u = np.ascontiguousarray(a, dtype=np.float32).view(np.uint32)
    r = (u + np.uint32(0x7FFF) + ((u >> np.uint32(16)) & np.uint32(1))) & np.uint32(
        0xFFFF0000
    )
    return r.view(np.float32)


def _to_bf16(a: np.ndarray):
    import ml_dtypes

    return np.ascontiguousarray(a).astype(ml_dtypes.bfloat16)


def _host_inputs(x: np.ndarray) -> list[dict]:
    xb = _round_bf16(x)  # [N, D] on the bf16 grid
    xt = np.ascontiguousarray(xb.T)  # [D, N]
    sqn = (xb.astype(np.float64) ** 2).sum(1).astype(np.float32)
    sqn_hi = _round_bf16(sqn)
    sqn_lo = _round_bf16(sqn - sqn_hi)
    ones = np.ones((1, N), np.float32)
    # -2*xb is exact (exponent shift), stays on the bf16 grid.
    b_full = np.concatenate(
        [-2.0 * xt, sqn_hi[None, :], sqn_lo[None, :], ones, ones], axis=0
    )  # [K, N]
    a_full = np.concatenate(
        [xt, ones, ones, sqn_hi[None, :], sqn_lo[None, :]], axis=0
    )  # [K, N]
    in_maps = []
    for c in range(NCORES):
        r0 = c * RPC
        bt = np.concatenate([b_full[:, r0:], b_full[:, :r0]], axis=1)[:, :BLOAD]
        in_maps.append(
            {
                "bt": _to_bf16(bt),
                "at": _to_bf16(a_full[:, r0 : r0 + RPC]),
            }
        )
    return in_maps


def _assemble(blocks: list[np.ndarray]) -> np.ndarray:
    """blocks[c]: [RPC, LB] fp16 d^2 bands -> full [N, N] fp32 distances."""
    full = np.empty((N, N), np.float32)
    HB = 128  # rows sharing one band start (1 row tile)
    for c in range(NCORES):
        blk = np.sqrt(np.maximum(blocks[c].astype(np.float32), 0.0))
        for half in range(RPC // HB):
            r0 = c * RPC + half * HB  # band's global col start == r0
            data = blk[half * HB : (half + 1) * HB]
            seg = min(LB, N - r0)
            full[r0 : r0 + HB, r0 : r0 + seg] = data[:, :seg]
            if seg < LB:
                full[r0 : r0 + HB, : LB - seg] = data[:, seg:]
    # Mirror the uncovered circular segment of each HB-row block from the
    # transpose (those entries are always device-covered).
    for b in range(N // HB):
        r0 = b * HB
        s = (r0 + LB) % N
        e = s + (N - LB)
        if e <= N:
            full[r0 : r0 + HB, s:e] = full[s:e, r0 : r0 + HB].T
        else:
            full[r0 : r0 + HB, s:N] = full[s:N, r0 : r0 + HB].T
            full[r0 : r0 + HB, : e - N] = full[: e - N, r0 : r0 + HB].T
    np.fill_diagonal(full, 0.0)
    return full


def _run(inputs, trace=False, trace_cores=None):
    from concourse.bass_utils import run_bass_kernel_spmd

    x = np.ascontiguousarray(np.asarray(inputs["x"], dtype=np.float32))
    assert x.shape == (N, D), x.shape
    res = run_bass_kernel_spmd(
        _get_nc(),
        _host_inputs(x),
        core_ids=list(range(NCORES)),
        trace=trace,
        trace_cores=trace_cores,
    )
    full = _assemble([r["out"] for r in res.results])
    return full, res


def kernel(**inputs) -> np.ndarray:
    full, _ = _run(inputs)
    return full
